# revision 1
# baseline (speedup 1.0000x reference)
"""FBPinn forward kernel for Trainium2 (8 NeuronCores, Bass/Tile).

The module computes y(x) = tanh(x) * sum_w [win_w(x)>1e-3] * win_w(x) * MLP_w(x)
for 1M scalar points x in [0,100) -- a fixed 1D function of x. Per core:
  1. evaluate the function at the 4097 knots of a uniform grid over the core's
     12.5-wide domain slice (32768 cells total) using the 30 tiny MLPs
     (block-diagonal-packed PE matmuls, tanh/sigmoid on ACT), masking windows
     exactly at each knot;
  2. assemble per-cell linear records (a0,b0,a1,b1,xsplit) -- two-sided at the
     54 win==1e-3 mask discontinuities so the jumps are reproduced exactly --
     entirely in SBUF, aligned so partition p owns cells [32p, 32p+32);
  3. points are packed (host side) into a (partition, cell)-aligned slot grid:
     cell c -> partition c//32, block c%32, S slots per cell. Interpolation is
     then pure elementwise DVE work with stride-0 broadcast reads of the
     records -- no gather at all.
Host shards points by domain across the 8 cores, packs slots, and un-permutes
the outputs. Piecewise-linear error on this grid is ~2e-6 absmax.
"""

import numpy as np

# ---------------- problem constants (hardcoded from the module spec) ----------
NW = 30
DOM0, DOM1 = 0.0, 100.0
OVERLAP = 0.25
NEURONS = 32
THRESH = 0.001
N = 1_000_000

NCORES = 8
P = 128                      # SBUF partitions
CPP = 24                     # cells per partition
C_LOC = P * CPP              # cells per core (4096)
DW = 12.5                    # per-core domain width
H = DW / C_LOC               # cell width (exact in fp32)
INVH = C_LOC / DW
NG = 3                       # window groups of 4 per core
NSLOT = 4 * NG               # window slots per core
KCHUNK = 512                 # knot columns per pipeline chunk
NKFULL = C_LOC // KCHUNK     # full chunks (6)
KCH_LAST = 128               # short final chunk (last knot + specials + pad)
NKCH = NKFULL + 1
KT = KCHUNK * NKFULL + KCH_LAST
NKNOT = C_LOC + 1            # real grid knots
NB = 16                      # straddle-boundary slots per core
SPEC0 = C_LOC + 8            # first special knot column
S_DEFAULT = 76               # point slots per cell
BIG = np.float32(1e30)


# ---------------- geometry (host, input-independent) --------------------------
def _partition_geom():
    width = (DOM1 - DOM0) / NW
    sub = np.zeros((NW, 2), np.float32)
    for i in range(NW):
        sub[i, 0] = DOM0 if i == 0 else DOM0 + (i - OVERLAP / 2) * width
        sub[i, 1] = DOM1 if i == NW - 1 else DOM0 + (i + 1 + OVERLAP / 2) * width
    means = (sub[:, 0] + sub[:, 1]) / 2
    std = (sub[:, 1] - sub[:, 0]) / 2
    mid = np.zeros(NW + 1, np.float32)
    mid[0] = sub[0, 0]
    mid[-1] = sub[-1, 1]
    for i in range(1, NW):
        mid[i] = (sub[i - 1, 1] + sub[i, 0]) / 2
    return means.astype(np.float32), std.astype(np.float32), mid.astype(np.float32)


def _win64(l, r, x):
    return 1.0 / (1 + np.exp(-(x - l))) / (1 + np.exp(x - r))


def _bisect64(l, r, lo, hi, rising):
    for _ in range(200):
        m = 0.5 * (lo + hi)
        if (_win64(l, r, m) < THRESH) == rising:
            lo = m
        else:
            hi = m
    return 0.5 * (lo + hi)


def _refine_flip_fp32(l32, r32, b64, rising):
    """Exact fp32 x where the reference's jax-fp32 predicate win(x)>1e-3 flips.
    Returns the smallest fp32 x at which the predicate equals its right-side
    state. Falls back to the float64 bisection value if jax is unavailable."""
    try:
        import jax
        import jax.numpy as jnp

        cpu = jax.devices("cpu")[0]
        lo = np.float32(b64 - 5e-5)
        hi = np.float32(b64 + 5e-5)
        xs = np.arange(lo.view(np.int32), hi.view(np.int32) + 1,
                       dtype=np.int32).view(np.float32)
        with jax.default_device(cpu):
            win = np.asarray(
                jax.nn.sigmoid(jnp.asarray(xs) - np.float32(l32))
                * jax.nn.sigmoid(-(jnp.asarray(xs) - np.float32(r32)))
            )
        pred = win > np.float32(THRESH)
        state = pred if rising else ~pred
        if not state.any() or state.all():
            return np.float32(b64)
        k = int(np.argmax(state))
        if not state[k:].all():
            return np.float32(b64)
        return xs[k]
    except Exception:
        return np.float32(b64)


_GEOM = None


def _geometry():
    global _GEOM
    if _GEOM is not None:
        return _GEOM
    means, std, mid = _partition_geom()
    ml = mid[:-1].astype(np.float64)
    mr = mid[1:].astype(np.float64)
    Lb = np.zeros(NW, np.float32)   # window-on lower bound (exact fp32 flip)
    Rb = np.zeros(NW, np.float32)   # window-off upper bound
    for w in range(NW):
        c = 0.5 * (ml[w] + mr[w])
        l64 = _bisect64(ml[w], mr[w], ml[w] - 30, c, rising=True)
        r64 = _bisect64(ml[w], mr[w], c, mr[w] + 30, rising=False)
        Lb[w] = _refine_flip_fp32(mid[w], mid[w + 1], l64, rising=True)
        Rb[w] = _refine_flip_fp32(mid[w], mid[w + 1], r64, rising=False)
    bnds = []
    for w in range(NW):
        if DOM0 < Lb[w] < DOM1:
            bnds.append(float(Lb[w]))
        if DOM0 < Rb[w] < DOM1:
            bnds.append(float(Rb[w]))
    bnds = np.sort(np.array(bnds, np.float64))
    _GEOM = (means, std, mid, Lb, Rb, bnds)
    return _GEOM




def _group_chunk_need():
    """need[ch][g]: does group g contribute anywhere in knot-chunk ch for ANY
    core? Computed from input-independent window geometry."""
    means, std, mid, Lb, Rb, bnds = _geometry()
    need = [[False] * NG for _ in range(NKCH)]
    for core in range(NCORES):
        base = DOM0 + core * DW
        act = [w for w in range(NW) if (Rb[w] > base) and (Lb[w] < base + DW)]
        for slot, w in enumerate(act):
            g = slot // 4
            lo, hi = float(Lb[w]) - base, float(Rb[w]) - base
            for ch in range(NKCH):
                c_lo = ch * KCHUNK * H
                c_hi = (ch + 1) * KCHUNK * H
                if ch == NKCH - 1:
                    c_hi = 1e30    # specials chunk: straddle x anywhere
                    c_lo = 0.0
                if hi > c_lo - 4 * H and lo < c_hi + 4 * H:
                    need[ch][g] = True
    return need

# ---------------- bass program (built once per S, SPMD across 8 cores) --------
_PROGS = {}


def _build_program(S):
    if S in _PROGS:
        return _PROGS[S]
    from concourse import bacc, bass, mybir, tile
    from concourse.bass import IndirectOffsetOnAxis

    f32 = mybir.dt.float32
    f32r = mybir.dt.float32r
    i32 = mybir.dt.int32
    u8 = mybir.dt.uint8
    Act = mybir.ActivationFunctionType
    Op = mybir.AluOpType

    M = CPP * S
    PBLK = 8                      # cell-blocks per point chunk
    PCH = PBLK * S                # point columns per chunk
    NPCH = CPP // PBLK

    nc = bacc.Bacc(None, target_bir_lowering=False)

    x_in = nc.declare_dram_parameter("x_pts", [P, M], f32, isOutput=False)
    base_in = nc.declare_dram_parameter("base_col", [P, 1], f32, isOutput=False)
    c0p_in = nc.declare_dram_parameter("c0p_col", [P, 1], f32, isOutput=False)
    sc1_in = nc.declare_dram_parameter("sc1", [P, NG], f32, isOutput=False)
    bi1_in = nc.declare_dram_parameter("bi1", [P, NG], f32, isOutput=False)
    w2_in = nc.declare_dram_parameter("w2blk", [P, P * NG], f32, isOutput=False)
    w3_in = nc.declare_dram_parameter("w3f", [P, NSLOT * NG], f32, isOutput=False)
    b2_in = nc.declare_dram_parameter("b2c", [P, NG], f32, isOutput=False)
    b3_in = nc.declare_dram_parameter("b3c", [NSLOT, 1], f32, isOutput=False)
    negl_in = nc.declare_dram_parameter("negl", [NSLOT, 1], f32, isOutput=False)
    rr_in = nc.declare_dram_parameter("rr", [NSLOT, 1], f32, isOutput=False)
    lb_in = nc.declare_dram_parameter("lbnd", [NSLOT, 1], f32, isOutput=False)
    rb_in = nc.declare_dram_parameter("rbnd", [NSLOT, 1], f32, isOutput=False)
    knots_in = nc.declare_dram_parameter("knotrep", [P, KT], f32, isOutput=False)
    k12_in = nc.declare_dram_parameter("knot12", [NSLOT, KT], f32, isOutput=False)
    xs_in = nc.declare_dram_parameter("xsplit_col", [P, CPP], f32, isOutput=False)
    itb_in = nc.declare_dram_parameter("invtb_col", [P, CPP], f32, isOutput=False)
    i1tb_in = nc.declare_dram_parameter("inv1mtb_col", [P, CPP], f32, isOutput=False)
    jl_in = nc.declare_dram_parameter("jlist", [NB, 1], i32, isOutput=False)
    wm_in = nc.declare_dram_parameter("wmask", [NSLOT, KT], f32, isOutput=False)
    on_in = nc.declare_dram_parameter("ones12", [NSLOT, 1], f32, isOutput=False)
    y_out = nc.declare_dram_parameter("y_out", [P, M], f32, isOutput=True)

    with tile.TileContext(nc) as tc:
        with (
            tc.tile_pool(name="const", bufs=1) as cpool,
            tc.tile_pool(name="work", bufs=2) as wpool,
            tc.tile_pool(name="pts", bufs=2) as ppool,
            tc.tile_pool(name="psum", bufs=2, space="PSUM") as psum,
            tc.tile_pool(name="dram", bufs=1, space="DRAM") as dpool,
        ):
            v_dram = dpool.tile([KT, 1], f32)            # knot values
            vm_dram = dpool.tile([C_LOC + NB, 1], f32)   # left-limit knot col
            vp_dram = dpool.tile([C_LOC + NB, 1], f32)   # right-limit knot col

            _eng = [nc.sync, nc.scalar, nc.gpsimd]
            _eng_i = [0]

            def load(handle, shape, tag, dtype=f32, eng=None):
                t = cpool.tile(shape, dtype, tag=tag)
                e = eng if eng is not None else _eng[_eng_i[0] % len(_eng)]
                _eng_i[0] += 1
                e.dma_start(out=t[:], in_=handle[:])
                return t

            xp = load(x_in, [P, M], "c_x", eng=nc.gpsimd)
            base_c = load(base_in, [P, 1], "c_base")
            c0p = load(c0p_in, [P, 1], "c_c0p")
            sc1 = load(sc1_in, [P, NG], "c_sc1")
            bi1 = load(bi1_in, [P, NG], "c_bi1")
            w2 = load(w2_in, [P, P * NG], "c_w2")
            w3 = load(w3_in, [P, NSLOT * NG], "c_w3")
            b2 = load(b2_in, [P, NG], "c_b2")
            b3 = load(b3_in, [NSLOT, 1], "c_b3")
            negl = load(negl_in, [NSLOT, 1], "c_negl")
            rr = load(rr_in, [NSLOT, 1], "c_rr")
            k12 = load(k12_in, [NSLOT, KT], "c_k12", eng=nc.scalar)
            knots = load(knots_in, [P, KT], "c_knots", eng=nc.sync)
            xs_c = load(xs_in, [P, CPP], "c_xs")
            itb = load(itb_in, [P, CPP], "c_itb")
            i1tb = load(i1tb_in, [P, CPP], "c_i1tb")
            jl = load(jl_in, [NB, 1], "c_jl", i32)
            wmask = load(wm_in, [NSLOT, KT], "c_wm", eng=nc.scalar)

            ones12 = load(on_in, [NSLOT, 1], "c_ones")
            jrow_i = cpool.tile([P, M], i32, tag="c_jri")
            nc.gpsimd.iota(
                jrow_i[:].rearrange("p (c s) -> p c s", c=CPP),
                pattern=[[1, CPP], [0, S]], channel_multiplier=0,
            )
            jrow = cpool.tile([P, M], f32, tag="c_jrf")
            nc.vector.tensor_copy(out=jrow[:], in_=jrow_i[:])

            # ---- phase B: knot values ----
            need = _group_chunk_need()
            # hoist all sigmoids + the win*mask product out of the chunk loop
            # (avoids per-chunk ACT table reloads between Tanh and Sigmoid)
            s1a = cpool.tile([NSLOT, KT], f32, tag="s1a")
            nc.scalar.activation(out=s1a[:], in_=k12[:],
                                 func=Act.Sigmoid, bias=negl[:], scale=1.0)
            s2a = cpool.tile([NSLOT, KT], f32, tag="s2a")
            nc.scalar.activation(out=s2a[:], in_=k12[:],
                                 func=Act.Sigmoid, bias=rr[:], scale=-1.0)
            wina = cpool.tile([NSLOT, KT], f32, tag="wina")
            nc.vector.tensor_mul(out=wina[:], in0=s1a[:], in1=s2a[:])
            nc.vector.tensor_mul(out=wina[:], in0=wina[:], in1=wmask[:])
            tha = cpool.tile([1, KT], f32, tag="tha")
            nc.scalar.activation(out=tha[:], in_=k12[0:1, :], func=Act.Tanh)
            for ch in range(NKCH):
                kw = KCHUNK if ch < NKFULL else KCH_LAST
                sl = slice(ch * KCHUNK, ch * KCHUNK + kw)
                xk = knots[:, sl]
                gs = [g for g in range(NG) if need[ch][g]]
                h2s = {}
                for g in gs:
                    h1 = wpool.tile([P, KCHUNK], f32, tag="h1")
                    nc.scalar.activation(
                        out=h1[:, :kw], in_=xk, func=Act.Tanh,
                        bias=bi1[:, g : g + 1], scale=sc1[:, g : g + 1],
                    )
                    h2p = psum.tile([P, KCHUNK], f32, tag="h2p")
                    nc.tensor.matmul(
                        out=h2p[:, :kw], lhsT=w2[:, g * P : (g + 1) * P],
                        rhs=h1[:, :kw], start=True, stop=True,
                    )
                    h2 = wpool.tile([P, KCHUNK], f32, tag=f"h2_{g}")
                    nc.scalar.activation(
                        out=h2[:, :kw], in_=h2p[:, :kw], func=Act.Tanh,
                        bias=b2[:, g : g + 1], scale=1.0,
                    )
                    h2s[g] = h2
                pre = psum.tile([NSLOT, KCHUNK], f32, tag="pre")
                for i, g in enumerate(gs):
                    nc.tensor.matmul(
                        out=pre[:, :kw],
                        lhsT=w3[:, g * NSLOT : (g + 1) * NSLOT],
                        rhs=h2s[g][:, :kw], start=(i == 0), stop=(i == len(gs) - 1),
                    )
                term = wpool.tile([NSLOT, KCHUNK], f32, tag="term")
                nc.vector.tensor_scalar(out=term[:, :kw], in0=pre[:, :kw],
                                        scalar1=b3[:], scalar2=None, op0=Op.add)
                nc.vector.tensor_mul(out=term[:, :kw], in0=term[:, :kw],
                                     in1=wina[:, sl])
                vp_ps = psum.tile([1, KCHUNK], f32, tag="vp")
                nc.tensor.matmul(out=vp_ps[:, :kw], lhsT=ones12[:],
                                 rhs=term[:, :kw], start=True, stop=True)
                vrow = wpool.tile([1, KCHUNK], f32, tag="vrow")
                nc.vector.tensor_mul(out=vrow[:, :kw], in0=vp_ps[:, :kw],
                                     in1=tha[:, sl])
                nc.sync.dma_start(out=v_dram[sl, 0], in_=vrow[:, :kw])

            # ---- phase C: per-cell records in SBUF ----
            # vm[j] = left-limit of v at cell j's right end (default v[j+1]);
            # vp[j] = right-limit of v at cell j's split (default v[j]).
            nc.sync.dma_start(out=vm_dram[0:C_LOC, 0], in_=v_dram[1 : C_LOC + 1, 0])
            nc.sync.dma_start(out=vp_dram[0:C_LOC, 0], in_=v_dram[0:C_LOC, 0])
            sp2 = wpool.tile([NB, 2], f32, tag="sp2")
            nc.sync.dma_start(out=sp2[:], in_=v_dram[SPEC0 : SPEC0 + 2 * NB, 0])
            nc.gpsimd.indirect_dma_start(
                out=vm_dram[:, :],
                out_offset=IndirectOffsetOnAxis(ap=jl[:, :1], axis=0),
                in_=sp2[:, 0:1], in_offset=None,
            )
            nc.gpsimd.indirect_dma_start(
                out=vp_dram[:, :],
                out_offset=IndirectOffsetOnAxis(ap=jl[:, :1], axis=0),
                in_=sp2[:, 1:2], in_offset=None,
            )
            u_lo = wpool.tile([P, CPP], f32, tag="ulo")
            nc.sync.dma_start(out=u_lo[:], in_=v_dram[0:C_LOC, 0])
            u_hi = wpool.tile([P, CPP], f32, tag="uhi")
            nc.sync.dma_start(out=u_hi[:], in_=v_dram[1 : C_LOC + 1, 0])
            vm = wpool.tile([P, CPP], f32, tag="vm")
            nc.sync.dma_start(out=vm[:], in_=vm_dram[0:C_LOC, 0])
            vpt = wpool.tile([P, CPP], f32, tag="vpt")
            nc.sync.dma_start(out=vpt[:], in_=vp_dram[0:C_LOC, 0])
            b0c = wpool.tile([P, CPP], f32, tag="b0c")
            nc.vector.tensor_sub(out=b0c[:], in0=vm[:], in1=u_lo[:])
            nc.vector.tensor_mul(out=b0c[:], in0=b0c[:], in1=itb[:])
            b1c = wpool.tile([P, CPP], f32, tag="b1c")
            nc.vector.tensor_sub(out=b1c[:], in0=u_hi[:], in1=vpt[:])
            nc.vector.tensor_mul(out=b1c[:], in0=b1c[:], in1=i1tb[:])
            a1c = wpool.tile([P, CPP], f32, tag="a1c")
            nc.vector.tensor_sub(out=a1c[:], in0=u_hi[:], in1=b1c[:])
            da = wpool.tile([P, CPP], f32, tag="da")
            nc.vector.tensor_sub(out=da[:], in0=a1c[:], in1=u_lo[:])
            db = wpool.tile([P, CPP], f32, tag="db")
            nc.vector.tensor_sub(out=db[:], in0=b1c[:], in1=b0c[:])

            # ---- phase D: per-point interpolation ----
            def bcast(tile_, bsl):
                return tile_[:, bsl].to_broadcast([P, PBLK, S])

            for ch in range(NPCH):
                psl = slice(ch * PCH, (ch + 1) * PCH)
                bsl = slice(ch * PBLK, (ch + 1) * PBLK)
                xc = xp[:, psl]
                d = ppool.tile([P, PCH], f32, tag="d")
                nc.vector.tensor_scalar(out=d[:], in0=xc, scalar1=base_c[:],
                                        scalar2=None, op0=Op.subtract)
                s = ppool.tile([P, PCH], f32, tag="s")
                nc.vector.tensor_scalar(out=s[:], in0=d[:], scalar1=float(INVH),
                                        scalar2=c0p[:], op0=Op.mult,
                                        op1=Op.subtract)
                t = ppool.tile([P, PCH], f32, tag="t")
                nc.vector.tensor_sub(out=t[:], in0=s[:], in1=jrow[:, psl])
                x3 = xc.rearrange("p (c s) -> p c s", c=PBLK)
                side = ppool.tile([P, PCH], f32, tag="side")
                s3 = side[:].rearrange("p (c s) -> p c s", c=PBLK)
                nc.vector.tensor_tensor(out=s3, in0=x3, in1=bcast(xs_c, bsl),
                                        op=Op.is_ge)
                # y = (b0 + side*db)*t + (a0 + side*da)
                bb = ppool.tile([P, PCH], f32, tag="bb")
                bb3 = bb[:].rearrange("p (c s) -> p c s", c=PBLK)
                nc.vector.tensor_tensor(out=bb3, in0=s3, in1=bcast(db, bsl),
                                        op=Op.mult)
                nc.vector.tensor_tensor(out=bb3, in0=bb3, in1=bcast(b0c, bsl),
                                        op=Op.add)
                aa = ppool.tile([P, PCH], f32, tag="aa")
                aa3 = aa[:].rearrange("p (c s) -> p c s", c=PBLK)
                nc.vector.tensor_tensor(out=aa3, in0=s3, in1=bcast(da, bsl),
                                        op=Op.mult)
                nc.vector.tensor_tensor(out=aa3, in0=aa3, in1=bcast(u_lo, bsl),
                                        op=Op.add)
                y = ppool.tile([P, PCH], f32, tag="y")
                nc.vector.tensor_mul(out=y[:], in0=bb[:], in1=t[:])
                nc.vector.tensor_add(out=y[:], in0=y[:], in1=aa[:])
                nc.sync.dma_start(out=y_out[:, psl], in_=y[:])

    nc.compile()
    _PROGS[S] = nc
    return nc


# ---------------- host-side input prep ----------------------------------------
def _fold_weights(core, W1, b1, W2, b2, W3, b3):
    means, std, mid, Lb, Rb, bnds = _geometry()
    base = DOM0 + core * DW
    act = [w for w in range(NW) if (Rb[w] > base) and (Lb[w] < base + DW)]
    assert len(act) <= NSLOT, f"core {core}: {len(act)} active windows"
    sc1 = np.zeros((P, NG), np.float32)
    bi1 = np.zeros((P, NG), np.float32)
    w2blk = np.zeros((P, P * NG), np.float32)
    w3f = np.zeros((P, NSLOT * NG), np.float32)
    b2c = np.zeros((P, NG), np.float32)
    b3c = np.zeros((NSLOT, 1), np.float32)
    negl = np.zeros((NSLOT, 1), np.float32)
    rr = np.zeros((NSLOT, 1), np.float32)
    lbc = np.full((NSLOT, 1), BIG, np.float32)
    rbc = np.full((NSLOT, 1), -BIG, np.float32)
    for slot, w in enumerate(act):
        g, s = divmod(slot, 4)
        rows = slice(32 * s, 32 * s + 32)
        w1r = W1[w, 0, :].astype(np.float64)
        sc1[rows, g] = (w1r / std[w]).astype(np.float32)
        bi1[rows, g] = (b1[w] - w1r * means[w] / std[w]).astype(np.float32)
        w2blk[rows, g * P + 32 * s : g * P + 32 * s + 32] = W2[w]
        w3f[rows, g * NSLOT + slot] = W3[w, :, 0]
        b2c[rows, g] = b2[w]
        b3c[slot, 0] = b3[w, 0]
        negl[slot, 0] = -mid[w]
        rr[slot, 0] = mid[w + 1]
        lbc[slot, 0] = np.nextafter(Lb[w], -np.inf)
        rbc[slot, 0] = Rb[w]
    return sc1, bi1, w2blk, w3f, b2c, b3c, negl, rr, lbc, rbc


def _core_tables(core):
    """Knot x-values and straddle-cell helper arrays for one core."""
    means, std, mid, Lb, Rb, bnds = _geometry()
    base = DOM0 + core * DW
    # pad knots equal the last real knot so pad-cell slopes are exactly 0
    knot_row = np.full(KT, np.float32(base + DW), np.float32)
    kidx = np.arange(NKNOT, dtype=np.float64)
    knot_row[:NKNOT] = (base + kidx * H).astype(np.float32)
    bl = [b for b in bnds if base <= b < base + DW]
    assert len(bl) <= NB
    jlist = np.zeros((NB, 1), np.int32)
    xsplit_col = np.full(C_LOC, BIG, np.float32)
    itb_col = np.ones(C_LOC, np.float32)
    i1tb_col = np.ones(C_LOC, np.float32)
    for k, b in enumerate(bl):
        bf = np.float32(b)
        j = int(np.floor((float(bf) - base) / H))
        assert 0 <= j < C_LOC
        tB = (float(bf) - (base + j * H)) / H
        tB = min(max(tB, 1e-7), 1 - 1e-7)
        jlist[k, 0] = j
        xsplit_col[j] = bf
        itb_col[j] = np.float32(1.0 / tB)
        i1tb_col[j] = np.float32(1.0 / (1.0 - tB))
        knot_row[SPEC0 + 2 * k] = np.nextafter(bf, np.float32(-np.inf))
        knot_row[SPEC0 + 2 * k + 1] = bf
    for k in range(len(bl), NB):
        jlist[k, 0] = C_LOC + k       # dummy scatter rows, never read back
    knotrep = np.broadcast_to(knot_row, (P, KT)).copy()
    knot12 = np.broadcast_to(knot_row, (NSLOT, KT)).copy()
    # window mask at every knot: (knot > nextbelow(Lb)) & (knot < Rb) per slot
    base2 = DOM0 + core * DW
    act = [w for w in range(NW) if (Rb[w] > base2) and (Lb[w] < base2 + DW)]
    wmask = np.zeros((NSLOT, KT), np.float32)
    for slot, w in enumerate(act):
        lbv = np.nextafter(Lb[w], -np.inf)
        wmask[slot] = ((knot_row > lbv) & (knot_row < Rb[w])).astype(np.float32)
    return (knotrep, knot12, xsplit_col.reshape(P, CPP), itb_col.reshape(P, CPP),
            i1tb_col.reshape(P, CPP), jlist, wmask)


def _prep_in_maps(inputs, S):
    x = np.asarray(inputs["x"], np.float32)
    W1 = np.asarray(inputs["W1"], np.float32)
    b1 = np.asarray(inputs["b1"], np.float32)
    W2 = np.asarray(inputs["W2"], np.float32)
    b2 = np.asarray(inputs["b2"], np.float32)
    W3 = np.asarray(inputs["W3"], np.float32)
    b3 = np.asarray(inputs["b3"], np.float32)
    M = CPP * S

    # global cell of each point, then slot position inside the padded grid
    cglob = np.minimum((x.astype(np.float64) * (1.0 / H)).astype(np.int64),
                       NCORES * C_LOC - 1)
    order = np.argsort(cglob, kind="stable")
    cs = cglob[order]
    cnt = np.bincount(cglob, minlength=NCORES * C_LOC)
    maxcnt = int(cnt.max())
    if maxcnt > S:
        raise OverflowError(maxcnt)
    starts = np.concatenate(([0], np.cumsum(cnt)))
    rank = np.arange(len(x)) - starts[cs]           # rank within own cell
    slot = cs * S + rank                            # global padded slot index

    in_maps = []
    for core in range(NCORES):
        base = np.float32(DOM0 + core * DW)
        # pad x with each cell's left-edge x so t~0 and y=a0 (finite, discarded)
        cellx = (base + np.arange(C_LOC, dtype=np.float64) * H).astype(np.float32)
        xpad = np.repeat(cellx, S)
        msk = (cs >= core * C_LOC) & (cs < (core + 1) * C_LOC)
        xpad[slot[msk] - core * C_LOC * S] = x[order[msk]]
        sc1, bi1, w2blk, w3f, b2c, b3c, negl, rr, lbc, rbc = _fold_weights(
            core, W1, b1, W2, b2, W3, b3)
        (knotrep, knot12, xsplit_col, itb_col, i1tb_col, jlist,
         wmask) = _core_tables(core)
        in_maps.append({
            "x_pts": xpad.reshape(P, M),
            "base_col": np.full((P, 1), base, np.float32),
            "c0p_col": (np.arange(P, dtype=np.float32) * CPP).reshape(P, 1),
            "sc1": sc1, "bi1": bi1, "w2blk": w2blk, "w3f": w3f,
            "b2c": b2c, "b3c": b3c, "negl": negl, "rr": rr,
            "lbnd": lbc, "rbnd": rbc,
            "knotrep": knotrep, "knot12": knot12, "xsplit_col": xsplit_col,
            "invtb_col": itb_col, "inv1mtb_col": i1tb_col,
            "jlist": jlist, "wmask": wmask,
            "ones12": np.ones((NSLOT, 1), np.float32),
        })
    return in_maps, order, slot


def _unpack(results, order, slot, n_total):
    allys = np.concatenate([r["y_out"].reshape(-1) for r in results])
    out = np.empty(n_total, np.float32)
    out[order] = allys[slot]
    return out


def kernel(**inputs) -> np.ndarray:
    from concourse.bass_utils import run_bass_kernel_spmd

    S = S_DEFAULT
    while True:
        try:
            in_maps, order, slot = _prep_in_maps(inputs, S)
            break
        except OverflowError as e:
            S = ((int(e.args[0]) + 11) // 8) * 8   # headroom, multiple of 8
    nc = _build_program(S)
    res = run_bass_kernel_spmd(nc, in_maps, list(range(NCORES)))
    return _unpack(res.results, order, slot, len(np.asarray(inputs["x"])))



# revision 6
# speedup vs baseline: 3.7113x; 3.7113x over previous
"""FBPinn forward kernel for Trainium2 (8 NeuronCores, Bass/Tile).

The module computes y(x) = tanh(x) * sum_w [win_w(x)>1e-3] * win_w(x) * MLP_w(x)
for 1M scalar points x in [0,100) -- a fixed 1D function of x. Tolerance is
rel 2e-2, so a piecewise-linear table on a coarse grid suffices (measured
~2e-3 absmax with 128 cells/core, dominated by interpolating through the
win>1e-3 mask jumps, which this kernel does NOT special-case).

Per core (12.5-wide domain slice, 128 cells, one cell per SBUF partition):
  1. phase B: evaluate the function at the 129 knots of a uniform grid using
     the <=12 active per-window MLPs (block-diagonal-packed PE matmuls, all
     activations are Tanh -- window sigmoids use sigmoid(z)=(1+tanh(z/2))/2 --
     so the ACT table never swaps; a dummy activation at t=0 prefetches it).
     Knot x values are generated on-chip (iota) and folded into activation
     scale/bias host-side: no knot tables are DMAed.
  2. phase C: transpose the [1,129] knot-value row into per-partition records
     via two trivial PE matmuls (lhsT=row, rhs=[1,1] ones), then build
     y = P_p*x + Q_p slope/offset records with a few [128,1] DVE ops.
  3. phase D: points are packed (host side) so partition p holds exactly the
     points of cell p (S slots, left-edge-padded). Interpolation is ONE fused
     tensor_scalar op per chunk: y = x*P_p + Q_p. 4 chunks overlap stores.
Host shards points by domain across the 8 cores, packs slots, and un-permutes
the outputs.
"""

import numpy as np

# ---------------- problem constants (hardcoded from the module spec) ----------
NW = 30
DOM0, DOM1 = 0.0, 100.0
OVERLAP = 0.25
NEURONS = 32
THRESH = 0.001
N = 1_000_000

NCORES = 8
P = 128                      # SBUF partitions
C_LOC = P                    # cells per core: one per partition
DW = 12.5                    # per-core domain width
H = DW / C_LOC               # cell width = 0.09765625 (exact in fp32)
INVH = C_LOC / DW
NG = 3                       # window groups of 4 per core
NSLOT = 4 * NG               # window slots per core
NK = C_LOC + 1               # knots per core (129)
NKP = 132                    # padded iota width
NCH = 4                      # phase-D point chunks
S_DEFAULT = 1104             # point slots per cell (max bin count 1084 + pad)
BIG = np.float32(1e30)

# const-pack layout ([P, CW] tensor): w2blk | sc1h | bi1b | b2c | w3f | xl
CW = P * NG + NG + NG + NG + NSLOT * NG + 1
O_W2 = 0
O_SC1 = P * NG
O_BI1 = O_SC1 + NG
O_B2 = O_BI1 + NG
O_W3 = O_B2 + NG
O_XL = O_W3 + NSLOT * NG
# slot-pack layout ([NSLOT, SW]): wmask(0.25*mask) | b3 | t1b | t2b | ones | -1
SW = NK + 5
O_WM = 0
O_B3 = NK
O_T1 = NK + 1
O_T2 = NK + 2
O_ON = NK + 3
O_NEG = NK + 4


# ---------------- geometry (host, input-independent) --------------------------
def _partition_geom():
    width = (DOM1 - DOM0) / NW
    sub = np.zeros((NW, 2), np.float32)
    for i in range(NW):
        sub[i, 0] = DOM0 if i == 0 else DOM0 + (i - OVERLAP / 2) * width
        sub[i, 1] = DOM1 if i == NW - 1 else DOM0 + (i + 1 + OVERLAP / 2) * width
    means = (sub[:, 0] + sub[:, 1]) / 2
    std = (sub[:, 1] - sub[:, 0]) / 2
    mid = np.zeros(NW + 1, np.float32)
    mid[0] = sub[0, 0]
    mid[-1] = sub[-1, 1]
    for i in range(1, NW):
        mid[i] = (sub[i - 1, 1] + sub[i, 0]) / 2
    return means.astype(np.float32), std.astype(np.float32), mid.astype(np.float32)


def _win64(l, r, x):
    return 1.0 / (1 + np.exp(-(x - l))) / (1 + np.exp(x - r))


def _bisect64(l, r, lo, hi, rising):
    for _ in range(200):
        m = 0.5 * (lo + hi)
        if (_win64(l, r, m) < THRESH) == rising:
            lo = m
        else:
            hi = m
    return 0.5 * (lo + hi)


def _refine_flip_fp32(l32, r32, b64, rising):
    """Exact fp32 x where the reference's jax-fp32 predicate win(x)>1e-3 flips.
    Returns the smallest fp32 x at which the predicate equals its right-side
    state. Falls back to the float64 bisection value if jax is unavailable."""
    try:
        import jax
        import jax.numpy as jnp

        cpu = jax.devices("cpu")[0]
        lo = np.float32(b64 - 5e-5)
        hi = np.float32(b64 + 5e-5)
        xs = np.arange(lo.view(np.int32), hi.view(np.int32) + 1,
                       dtype=np.int32).view(np.float32)
        with jax.default_device(cpu):
            win = np.asarray(
                jax.nn.sigmoid(jnp.asarray(xs) - np.float32(l32))
                * jax.nn.sigmoid(-(jnp.asarray(xs) - np.float32(r32)))
            )
        pred = win > np.float32(THRESH)
        state = pred if rising else ~pred
        if not state.any() or state.all():
            return np.float32(b64)
        k = int(np.argmax(state))
        if not state[k:].all():
            return np.float32(b64)
        return xs[k]
    except Exception:
        return np.float32(b64)


_GEOM = None


def _geometry():
    global _GEOM
    if _GEOM is not None:
        return _GEOM
    means, std, mid = _partition_geom()
    ml = mid[:-1].astype(np.float64)
    mr = mid[1:].astype(np.float64)
    Lb = np.zeros(NW, np.float32)   # window-on lower bound (exact fp32 flip)
    Rb = np.zeros(NW, np.float32)   # window-off upper bound
    for w in range(NW):
        c = 0.5 * (ml[w] + mr[w])
        l64 = _bisect64(ml[w], mr[w], ml[w] - 30, c, rising=True)
        r64 = _bisect64(ml[w], mr[w], c, mr[w] + 30, rising=False)
        Lb[w] = _refine_flip_fp32(mid[w], mid[w + 1], l64, rising=True)
        Rb[w] = _refine_flip_fp32(mid[w], mid[w + 1], r64, rising=False)
    _GEOM = (means, std, mid, Lb, Rb)
    return _GEOM


def _active_windows(core):
    means, std, mid, Lb, Rb = _geometry()
    base = DOM0 + core * DW
    return [w for w in range(NW) if (Rb[w] > base) and (Lb[w] < base + DW)]


# ---------------- bass program (built once per S, SPMD across 8 cores) --------
_PROGS = {}


def _build_program(S):
    if S in _PROGS:
        return _PROGS[S]
    from concourse import bacc, mybir, tile

    f32 = mybir.dt.float32
    i32 = mybir.dt.int32
    Act = mybir.ActivationFunctionType
    Op = mybir.AluOpType

    assert S % NCH == 0
    CH = S // NCH

    nc = bacc.Bacc(None, target_bir_lowering=False)

    x_in = nc.declare_dram_parameter("x_pts", [P, S], f32, isOutput=False)
    cp_in = nc.declare_dram_parameter("cpack", [P, CW], f32, isOutput=False)
    sp_in = nc.declare_dram_parameter("spack", [NSLOT, SW], f32, isOutput=False)
    y_out = nc.declare_dram_parameter("y_out", [P, S], f32, isOutput=True)

    with tile.TileContext(nc) as tc:
        with (
            tc.tile_pool(name="const", bufs=1) as cpool,
            tc.tile_pool(name="work", bufs=2) as wpool,
            tc.tile_pool(name="pts", bufs=2) as ppool,
            tc.tile_pool(name="psum", bufs=1, space="PSUM") as psum,
        ):
            # ---- prefetch the Tanh ACT table behind the const DMAs ----
            dmy = cpool.tile([1, 4], f32, tag="dmy")
            nc.vector.memset(dmy[:], 0.0)
            dmy2 = cpool.tile([1, 4], f32, tag="dmy2")
            nc.scalar.activation(out=dmy2[:], in_=dmy[:], func=Act.Tanh)

            # ---- input DMAs: consts on sync ring, x chunks spread ----
            cpk = cpool.tile([P, CW], f32, tag="c_cpk")
            nc.sync.dma_start(out=cpk[:], in_=cp_in[:])
            spk = cpool.tile([NSLOT, SW], f32, tag="c_spk")
            nc.sync.dma_start(out=spk[:], in_=sp_in[:])
            xp = cpool.tile([P, S], f32, tag="c_x")
            xeng = [nc.sync, nc.sync, nc.gpsimd, nc.scalar]
            for ch in range(NCH):
                sl = slice(ch * CH, (ch + 1) * CH)
                xeng[ch % len(xeng)].dma_start(out=xp[:, sl], in_=x_in[:, sl])

            w2 = cpk[:, O_W2:O_W2 + P * NG]
            sc1h = cpk[:, O_SC1:O_SC1 + NG]
            bi1b = cpk[:, O_BI1:O_BI1 + NG]
            b2c = cpk[:, O_B2:O_B2 + NG]
            w3f = cpk[:, O_W3:O_W3 + NSLOT * NG]
            xl = cpk[:, O_XL:O_XL + 1]
            wm = spk[:, O_WM:O_WM + NK]
            b3c = spk[:, O_B3:O_B3 + 1]
            t1b = spk[:, O_T1:O_T1 + 1]
            t2b = spk[:, O_T2:O_T2 + 1]
            ones12 = spk[:, O_ON:O_ON + 1]
            negone = spk[:, O_NEG:O_NEG + 1]

            # ---- knot index row (on-chip, value = column index) ----
            jri = cpool.tile([P, NKP], i32, tag="c_jri")
            nc.gpsimd.iota(jri[:], pattern=[[1, NKP]], channel_multiplier=0)
            krow = cpool.tile([P, NKP], f32, tag="c_krf")
            nc.vector.tensor_copy(out=krow[:], in_=jri[:])

            # ---- phase B: knot values (single chunk, all-Tanh) ----
            h2s = []
            for g in range(NG):
                h1 = wpool.tile([P, NK], f32, tag=f"h1_{g}")
                nc.scalar.activation(out=h1[:], in_=krow[:, :NK], func=Act.Tanh,
                                     bias=bi1b[:, g:g + 1], scale=sc1h[:, g:g + 1])
                h2p = psum.tile([P, NK], f32, tag=f"h2p_{g}")
                nc.tensor.matmul(out=h2p[:], lhsT=w2[:, g * P:(g + 1) * P],
                                 rhs=h1[:], start=True, stop=True)
                h2 = wpool.tile([P, NK], f32, tag=f"h2_{g}")
                nc.scalar.activation(out=h2[:], in_=h2p[:], func=Act.Tanh,
                                     bias=b2c[:, g:g + 1], scale=1.0)
                h2s.append(h2)
            pre = psum.tile([NSLOT, NK], f32, tag="pre")
            for g in range(NG):
                nc.tensor.matmul(out=pre[:], lhsT=w3f[:, g * NSLOT:(g + 1) * NSLOT],
                                 rhs=h2s[g][:], start=(g == 0), stop=(g == NG - 1))
            # window: win = 0.25*(1+tanh((x-l)/2))*(1+tanh((r-x)/2)) * mask
            # (0.25*mask is folded into wm host-side)
            tha = wpool.tile([1, NK], f32, tag="tha")
            nc.scalar.activation(out=tha[:], in_=krow[0:1, :NK], func=Act.Tanh,
                                 scale=float(H), bias=xl[0:1, :])
            t1 = wpool.tile([NSLOT, NK], f32, tag="t1")
            nc.scalar.activation(out=t1[:], in_=krow[0:NSLOT, :NK], func=Act.Tanh,
                                 scale=float(H / 2), bias=t1b[:])
            t2 = wpool.tile([NSLOT, NK], f32, tag="t2")
            nc.scalar.activation(out=t2[:], in_=krow[0:NSLOT, :NK], func=Act.Tanh,
                                 scale=float(-H / 2), bias=t2b[:])
            wa = wpool.tile([NSLOT, NK], f32, tag="wa")
            nc.vector.tensor_scalar(out=wa[:], in0=t1[:], scalar1=1.0,
                                    scalar2=None, op0=Op.add)
            wb = wpool.tile([NSLOT, NK], f32, tag="wb")
            nc.vector.tensor_scalar(out=wb[:], in0=t2[:], scalar1=1.0,
                                    scalar2=None, op0=Op.add)
            win = wpool.tile([NSLOT, NK], f32, tag="win")
            nc.vector.tensor_mul(out=win[:], in0=wa[:], in1=wb[:])
            nc.vector.tensor_mul(out=win[:], in0=win[:], in1=wm[:])
            term = wpool.tile([NSLOT, NK], f32, tag="term")
            nc.vector.tensor_scalar(out=term[:], in0=pre[:], scalar1=b3c[:],
                                    scalar2=None, op0=Op.add)
            nc.vector.tensor_mul(out=term[:], in0=term[:], in1=win[:])
            vps = psum.tile([1, NK], f32, tag="vps")
            nc.tensor.matmul(out=vps[:], lhsT=ones12[:], rhs=term[:],
                             start=True, stop=True)
            vrow = wpool.tile([1, NK], f32, tag="vrow")
            nc.vector.tensor_mul(out=vrow[:], in0=vps[:], in1=tha[:])

            # ---- phase C: transpose knot row via PE, build P/Q records ----
            ulo = psum.tile([P, 1], f32, tag="ulo")
            nc.tensor.matmul(out=ulo[:], lhsT=vrow[:, 0:P], rhs=ones12[0:1, 0:1],
                             start=True, stop=True)
            dv = psum.tile([P, 1], f32, tag="dv")
            nc.tensor.matmul(out=dv[:], lhsT=vrow[:, 1:P + 1],
                             rhs=ones12[0:1, 0:1], start=True, stop=False)
            nc.tensor.matmul(out=dv[:], lhsT=vrow[:, 0:P],
                             rhs=negone[0:1, 0:1], start=False, stop=True)
            pc = wpool.tile([P, 1], f32, tag="pc")
            nc.vector.tensor_scalar(out=pc[:], in0=dv[:], scalar1=float(INVH),
                                    scalar2=None, op0=Op.mult)
            qt = wpool.tile([P, 1], f32, tag="qt")
            nc.vector.tensor_scalar(out=qt[:], in0=pc[:], scalar1=xl[:],
                                    scalar2=None, op0=Op.mult)
            qc = wpool.tile([P, 1], f32, tag="qc")
            nc.vector.tensor_sub(out=qc[:], in0=ulo[:], in1=qt[:])

            # ---- phase D: per-point interpolation, one fused op per chunk ----
            oeng = [nc.sync, nc.scalar]
            for ch in range(NCH):
                sl = slice(ch * CH, (ch + 1) * CH)
                y = ppool.tile([P, CH], f32, tag="y")
                nc.vector.tensor_scalar(out=y[:], in0=xp[:, sl], scalar1=pc[:],
                                        scalar2=qc[:], op0=Op.mult, op1=Op.add)
                oeng[ch % 2].dma_start(out=y_out[:, sl], in_=y[:])

    nc.compile()
    _PROGS[S] = nc
    return nc


# ---------------- host-side input prep ----------------------------------------
def _fold_weights(core, W1, b1, W2, b2, W3, b3):
    means, std, mid, Lb, Rb = _geometry()
    base = DOM0 + core * DW
    act = _active_windows(core)
    assert len(act) <= NSLOT, f"core {core}: {len(act)} active windows"
    cpack = np.zeros((P, CW), np.float32)
    spack = np.zeros((NSLOT, SW), np.float32)
    sc1h = np.zeros((P, NG), np.float64)
    bi1b = np.zeros((P, NG), np.float64)
    for slot, w in enumerate(act):
        g, s = divmod(slot, 4)
        rows = slice(32 * s, 32 * s + 32)
        w1r = W1[w, 0, :].astype(np.float64)
        sc1h[rows, g] = w1r / std[w] * H
        bi1b[rows, g] = b1[w] + w1r * (base - means[w]) / std[w]
        cpack[rows, O_W2 + g * P + 32 * s:O_W2 + g * P + 32 * s + 32] = W2[w]
        cpack[rows, O_W3 + g * NSLOT + slot] = W3[w, :, 0]
        cpack[rows, O_B2 + g] = b2[w]
        spack[slot, O_B3] = b3[w, 0]
        spack[slot, O_T1] = np.float32((base - np.float64(mid[w])) / 2.0)
        spack[slot, O_T2] = np.float32((np.float64(mid[w + 1]) - base) / 2.0)
    cpack[:, O_SC1:O_SC1 + NG] = sc1h.astype(np.float32)
    cpack[:, O_BI1:O_BI1 + NG] = bi1b.astype(np.float32)
    # cell-left x per partition (exact in fp32)
    cellx = (np.float64(base) + np.arange(P, dtype=np.float64) * H).astype(np.float32)
    cpack[:, O_XL] = cellx
    # window mask at knots (0.25 factor of the tanh-sigmoid identity folded in)
    kx = (np.float64(base) + np.arange(NK, dtype=np.float64) * H).astype(np.float32)
    for slot, w in enumerate(act):
        lbv = np.nextafter(Lb[w], -np.inf)
        spack[slot, O_WM:O_WM + NK] = 0.25 * ((kx > lbv) & (kx < Rb[w]))
    spack[:, O_ON] = 1.0
    spack[:, O_NEG] = -1.0
    # inactive slots: park the window far away so tanh args stay finite
    for slot in range(len(act), NSLOT):
        spack[slot, O_T1] = -1e4
        spack[slot, O_T2] = -1e4
    return cpack, spack


def _prep_in_maps(inputs, S):
    x = np.asarray(inputs["x"], np.float32)
    W1 = np.asarray(inputs["W1"], np.float32)
    b1 = np.asarray(inputs["b1"], np.float32)
    W2 = np.asarray(inputs["W2"], np.float32)
    b2 = np.asarray(inputs["b2"], np.float32)
    W3 = np.asarray(inputs["W3"], np.float32)
    b3 = np.asarray(inputs["b3"], np.float32)

    ncell = NCORES * C_LOC
    cglob = np.minimum((x.astype(np.float64) * (1.0 / H)).astype(np.int64),
                       ncell - 1)
    order = np.argsort(cglob, kind="stable")
    cs = cglob[order]
    cnt = np.bincount(cglob, minlength=ncell)
    maxcnt = int(cnt.max())
    if maxcnt > S:
        raise OverflowError(maxcnt)
    starts = np.concatenate(([0], np.cumsum(cnt)))
    rank = np.arange(len(x)) - starts[cs]           # rank within own cell
    slot = cs * S + rank                            # global padded slot index

    in_maps = []
    for core in range(NCORES):
        base = np.float32(DOM0 + core * DW)
        # pad x with each cell's left-edge x (finite y, discarded)
        cellx = (np.float64(base)
                 + np.arange(C_LOC, dtype=np.float64) * H).astype(np.float32)
        xpad = np.repeat(cellx, S)
        msk = (cs >= core * C_LOC) & (cs < (core + 1) * C_LOC)
        xpad[slot[msk] - core * C_LOC * S] = x[order[msk]]
        cpack, spack = _fold_weights(core, W1, b1, W2, b2, W3, b3)
        in_maps.append({
            "x_pts": xpad.reshape(P, S),
            "cpack": cpack,
            "spack": spack,
        })
    return in_maps, order, slot


def _unpack(results, order, slot, n_total):
    allys = np.concatenate([r["y_out"].reshape(-1) for r in results])
    out = np.empty(n_total, np.float32)
    out[order] = allys[slot]
    return out


def kernel(**inputs) -> np.ndarray:
    from concourse.bass_utils import run_bass_kernel_spmd

    S = S_DEFAULT
    while True:
        try:
            in_maps, order, slot = _prep_in_maps(inputs, S)
            break
        except OverflowError as e:
            S = ((int(e.args[0]) + 19) // 16) * 16   # headroom, multiple of 16
    nc = _build_program(S)
    res = run_bass_kernel_spmd(nc, in_maps, list(range(NCORES)))
    return _unpack(res.results, order, slot, len(np.asarray(inputs["x"])))


# revision 8
# speedup vs baseline: 3.8222x; 1.0299x over previous
"""FBPinn forward kernel for Trainium2 (8 NeuronCores, Bass/Tile).

The module computes y(x) = tanh(x) * sum_w [win_w(x)>1e-3] * win_w(x) * MLP_w(x)
for 1M scalar points x in [0,100) -- a fixed 1D function of x. Tolerance is
rel 2e-2, so a piecewise-linear table on a coarse grid suffices (measured
~2e-3 absmax with 128 cells/core, dominated by interpolating through the
win>1e-3 mask jumps, which this kernel does NOT special-case).

Per core (12.5-wide domain slice, 128 cells, one cell per SBUF partition):
  1. phase B: evaluate the function at the 129 knots of a uniform grid using
     the <=12 active per-window MLPs (block-diagonal-packed PE matmuls, all
     activations are Tanh -- window sigmoids use sigmoid(z)=(1+tanh(z/2))/2 --
     so the ACT table never swaps; a dummy activation at t=0 prefetches it).
     Knot x values are generated on-chip (iota) and folded into activation
     scale/bias host-side: no knot tables are DMAed.
  2. phase C: the final sum-over-slots matmul is done twice with lhsT=term
     shifted by one knot, yielding the knot values directly in partition-major
     layout ([128,1] columns): cell records u_lo/dv come out with 3 tiny DVE
     ops and zero DRAM roundtrips or transposes.
  3. phase D: points are packed (host side) so partition p holds exactly the
     points of cell p (S slots). The host sends t=(x-cell_left)/h as fp16;
     interpolation is ONE fused tensor_scalar per chunk: y = t*dv_p + ulo_p,
     written back as fp16 (host upcasts). 2 input chunks, 4 output chunks.
Host shards points by domain across the 8 cores, packs slots, and un-permutes
the outputs.
"""

import numpy as np

# ---------------- problem constants (hardcoded from the module spec) ----------
NW = 30
DOM0, DOM1 = 0.0, 100.0
OVERLAP = 0.25
NEURONS = 32
THRESH = 0.001
N = 1_000_000

NCORES = 8
P = 128                      # SBUF partitions
C_LOC = P                    # cells per core: one per partition
DW = 12.5                    # per-core domain width
H = DW / C_LOC               # cell width = 0.09765625 (exact in fp32)
INVH = C_LOC / DW
NG = 3                       # window groups of 4 per core
NSLOT = 4 * NG               # window slots per core
NK = C_LOC + 1               # knots per core (129)
NKP = 132                    # padded iota width
NCHI = 2                     # phase-D input chunks
NCHO = 4                     # phase-D output chunks
S_DEFAULT = 1104             # point slots per cell (max bin count 1084 + pad)
BIG = np.float32(1e30)

# small const-pack layout ([P, CSW]): sc1h | bi1b | b2c | w3f | xl | xr
CSW = NG + NG + NG + NSLOT * NG + 2
O_SC1 = 0
O_BI1 = O_SC1 + NG
O_B2 = O_BI1 + NG
O_W3 = O_B2 + NG
O_XL = O_W3 + NSLOT * NG
O_XR = O_XL + 1
# slot-pack layout ([NSLOT, SW]): wmask(0.25*mask) | b3 | t1b | t2b | ones
SW = NK + 4
O_WM = 0
O_B3 = NK
O_T1 = NK + 1
O_T2 = NK + 2
O_ON = NK + 3


# ---------------- geometry (host, input-independent) --------------------------
def _partition_geom():
    width = (DOM1 - DOM0) / NW
    sub = np.zeros((NW, 2), np.float32)
    for i in range(NW):
        sub[i, 0] = DOM0 if i == 0 else DOM0 + (i - OVERLAP / 2) * width
        sub[i, 1] = DOM1 if i == NW - 1 else DOM0 + (i + 1 + OVERLAP / 2) * width
    means = (sub[:, 0] + sub[:, 1]) / 2
    std = (sub[:, 1] - sub[:, 0]) / 2
    mid = np.zeros(NW + 1, np.float32)
    mid[0] = sub[0, 0]
    mid[-1] = sub[-1, 1]
    for i in range(1, NW):
        mid[i] = (sub[i - 1, 1] + sub[i, 0]) / 2
    return means.astype(np.float32), std.astype(np.float32), mid.astype(np.float32)


def _win64(l, r, x):
    return 1.0 / (1 + np.exp(-(x - l))) / (1 + np.exp(x - r))


def _bisect64(l, r, lo, hi, rising):
    for _ in range(200):
        m = 0.5 * (lo + hi)
        if (_win64(l, r, m) < THRESH) == rising:
            lo = m
        else:
            hi = m
    return 0.5 * (lo + hi)


def _refine_flip_fp32(l32, r32, b64, rising):
    """Exact fp32 x where the reference's jax-fp32 predicate win(x)>1e-3 flips.
    Returns the smallest fp32 x at which the predicate equals its right-side
    state. Falls back to the float64 bisection value if jax is unavailable."""
    try:
        import jax
        import jax.numpy as jnp

        cpu = jax.devices("cpu")[0]
        lo = np.float32(b64 - 5e-5)
        hi = np.float32(b64 + 5e-5)
        xs = np.arange(lo.view(np.int32), hi.view(np.int32) + 1,
                       dtype=np.int32).view(np.float32)
        with jax.default_device(cpu):
            win = np.asarray(
                jax.nn.sigmoid(jnp.asarray(xs) - np.float32(l32))
                * jax.nn.sigmoid(-(jnp.asarray(xs) - np.float32(r32)))
            )
        pred = win > np.float32(THRESH)
        state = pred if rising else ~pred
        if not state.any() or state.all():
            return np.float32(b64)
        k = int(np.argmax(state))
        if not state[k:].all():
            return np.float32(b64)
        return xs[k]
    except Exception:
        return np.float32(b64)


_GEOM = None


def _geometry():
    global _GEOM
    if _GEOM is not None:
        return _GEOM
    means, std, mid = _partition_geom()
    ml = mid[:-1].astype(np.float64)
    mr = mid[1:].astype(np.float64)
    Lb = np.zeros(NW, np.float32)   # window-on lower bound (exact fp32 flip)
    Rb = np.zeros(NW, np.float32)   # window-off upper bound
    for w in range(NW):
        c = 0.5 * (ml[w] + mr[w])
        l64 = _bisect64(ml[w], mr[w], ml[w] - 30, c, rising=True)
        r64 = _bisect64(ml[w], mr[w], c, mr[w] + 30, rising=False)
        Lb[w] = _refine_flip_fp32(mid[w], mid[w + 1], l64, rising=True)
        Rb[w] = _refine_flip_fp32(mid[w], mid[w + 1], r64, rising=False)
    _GEOM = (means, std, mid, Lb, Rb)
    return _GEOM


def _active_windows(core):
    means, std, mid, Lb, Rb = _geometry()
    base = DOM0 + core * DW
    return [w for w in range(NW) if (Rb[w] > base) and (Lb[w] < base + DW)]


# ---------------- bass program (built once per S, SPMD across 8 cores) --------
_PROGS = {}


def _build_program(S):
    if S in _PROGS:
        return _PROGS[S]
    from concourse import bacc, mybir, tile

    f32 = mybir.dt.float32
    f16 = mybir.dt.float16
    i32 = mybir.dt.int32
    Act = mybir.ActivationFunctionType
    Op = mybir.AluOpType

    assert S % (NCHI * NCHO) == 0
    CHI = S // NCHI
    CHO = S // NCHO

    nc = bacc.Bacc(None, target_bir_lowering=False)

    t_in = nc.declare_dram_parameter("t_pts", [P, S], f16, isOutput=False)
    cs_in = nc.declare_dram_parameter("cspack", [P, CSW], f32, isOutput=False)
    w2_in = nc.declare_dram_parameter("w2pack", [P, P * NG], f32, isOutput=False)
    sp_in = nc.declare_dram_parameter("spack", [NSLOT, SW], f32, isOutput=False)
    y_out = nc.declare_dram_parameter("y_out", [P, S], f16, isOutput=True)

    with tile.TileContext(nc) as tc:
        with (
            tc.tile_pool(name="const", bufs=1) as cpool,
            tc.tile_pool(name="work", bufs=2) as wpool,
            tc.tile_pool(name="pts", bufs=4) as ppool,
            tc.tile_pool(name="psum", bufs=1, space="PSUM") as psum,
        ):
            # ---- prefetch the Tanh ACT table behind the const DMAs ----
            dmy = cpool.tile([1, 4], f32, tag="dmy")
            nc.vector.memset(dmy[:], 0.0)
            dmy2 = cpool.tile([1, 4], f32, tag="dmy2")
            nc.scalar.activation(out=dmy2[:], in_=dmy[:], func=Act.Tanh)

            # ---- input DMAs: small consts first, then big streams ----
            csk = cpool.tile([P, CSW], f32, tag="c_csk")
            nc.sync.dma_start(out=csk[:], in_=cs_in[:])
            spk = cpool.tile([NSLOT, SW], f32, tag="c_spk")
            nc.sync.dma_start(out=spk[:], in_=sp_in[:])
            w2 = cpool.tile([P, P * NG], f32, tag="c_w2")
            nc.sync.dma_start(out=w2[:], in_=w2_in[:])
            tp = cpool.tile([P, S], f16, tag="c_t")
            teng = [nc.scalar, nc.gpsimd]
            for ch in range(NCHI):
                sl = slice(ch * CHI, (ch + 1) * CHI)
                teng[ch % len(teng)].dma_start(out=tp[:, sl], in_=t_in[:, sl])

            sc1h = csk[:, O_SC1:O_SC1 + NG]
            bi1b = csk[:, O_BI1:O_BI1 + NG]
            b2c = csk[:, O_B2:O_B2 + NG]
            w3f = csk[:, O_W3:O_W3 + NSLOT * NG]
            xl = csk[:, O_XL:O_XL + 1]
            xr = csk[:, O_XR:O_XR + 1]
            wm = spk[:, O_WM:O_WM + NK]
            b3c = spk[:, O_B3:O_B3 + 1]
            t1b = spk[:, O_T1:O_T1 + 1]
            t2b = spk[:, O_T2:O_T2 + 1]
            ones12 = spk[:, O_ON:O_ON + 1]

            # ---- knot index row (on-chip, value = column index) ----
            jri = cpool.tile([P, NKP], i32, tag="c_jri")
            nc.gpsimd.iota(jri[:], pattern=[[1, NKP]], channel_multiplier=0)
            krow = cpool.tile([P, NKP], f32, tag="c_krf")
            nc.vector.tensor_copy(out=krow[:], in_=jri[:])

            # ---- phase B: knot values (single chunk, all-Tanh) ----
            h2s = []
            for g in range(NG):
                h1 = wpool.tile([P, NK], f32, tag=f"h1_{g}")
                nc.scalar.activation(out=h1[:], in_=krow[:, :NK], func=Act.Tanh,
                                     bias=bi1b[:, g:g + 1], scale=sc1h[:, g:g + 1])
                h2p = psum.tile([P, NK], f32, tag=f"h2p_{g}")
                nc.tensor.matmul(out=h2p[:], lhsT=w2[:, g * P:(g + 1) * P],
                                 rhs=h1[:], start=True, stop=True)
                h2 = wpool.tile([P, NK], f32, tag=f"h2_{g}")
                nc.scalar.activation(out=h2[:], in_=h2p[:], func=Act.Tanh,
                                     bias=b2c[:, g:g + 1], scale=1.0)
                h2s.append(h2)
            pre = psum.tile([NSLOT, NK], f32, tag="pre")
            for g in range(NG):
                nc.tensor.matmul(out=pre[:], lhsT=w3f[:, g * NSLOT:(g + 1) * NSLOT],
                                 rhs=h2s[g][:], start=(g == 0), stop=(g == NG - 1))
            # window: win = 0.25*(1+tanh((x-l)/2))*(1+tanh((r-x)/2)) * mask
            # (0.25*mask is folded into wm host-side)
            t1 = wpool.tile([NSLOT, NK], f32, tag="t1")
            nc.scalar.activation(out=t1[:], in_=krow[0:NSLOT, :NK], func=Act.Tanh,
                                 scale=float(H / 2), bias=t1b[:])
            t2 = wpool.tile([NSLOT, NK], f32, tag="t2")
            nc.scalar.activation(out=t2[:], in_=krow[0:NSLOT, :NK], func=Act.Tanh,
                                 scale=float(-H / 2), bias=t2b[:])
            wbm = wpool.tile([NSLOT, NK], f32, tag="wbm")
            nc.vector.scalar_tensor_tensor(out=wbm[:], in0=t2[:], scalar=1.0,
                                           in1=wm[:], op0=Op.add, op1=Op.mult)
            win = wpool.tile([NSLOT, NK], f32, tag="win")
            nc.vector.scalar_tensor_tensor(out=win[:], in0=t1[:], scalar=1.0,
                                           in1=wbm[:], op0=Op.add, op1=Op.mult)
            term = wpool.tile([NSLOT, NK], f32, tag="term")
            nc.vector.scalar_tensor_tensor(out=term[:], in0=pre[:], scalar=b3c[:],
                                           in1=win[:], op0=Op.add, op1=Op.mult)

            # ---- phase C: partition-major knot sums + cell records ----
            vlo = psum.tile([P, 1], f32, tag="vlo")
            nc.tensor.matmul(out=vlo[:], lhsT=term[:, 0:P], rhs=ones12[:],
                             start=True, stop=True)
            vhi = psum.tile([P, 1], f32, tag="vhi")
            nc.tensor.matmul(out=vhi[:], lhsT=term[:, 1:P + 1], rhs=ones12[:],
                             start=True, stop=True)
            thlo = wpool.tile([P, 1], f32, tag="thlo")
            nc.scalar.activation(out=thlo[:], in_=xl[:], func=Act.Tanh)
            thhi = wpool.tile([P, 1], f32, tag="thhi")
            nc.scalar.activation(out=thhi[:], in_=xr[:], func=Act.Tanh)
            ulos = wpool.tile([P, 1], f32, tag="ulos")
            nc.vector.tensor_mul(out=ulos[:], in0=vlo[:], in1=thlo[:])
            uhis = wpool.tile([P, 1], f32, tag="uhis")
            nc.vector.tensor_mul(out=uhis[:], in0=vhi[:], in1=thhi[:])
            dvs = wpool.tile([P, 1], f32, tag="dvs")
            nc.vector.tensor_sub(out=dvs[:], in0=uhis[:], in1=ulos[:])

            # ---- phase D: y = t*dv + ulo, fp16 in/out, one fused op/chunk ----
            oeng = [nc.sync, nc.scalar]
            for ch in range(NCHO):
                sl = slice(ch * CHO, (ch + 1) * CHO)
                y = ppool.tile([P, CHO], f16, tag="y")
                nc.vector.tensor_scalar(out=y[:], in0=tp[:, sl], scalar1=dvs[:],
                                        scalar2=ulos[:], op0=Op.mult, op1=Op.add)
                oeng[ch % 2].dma_start(out=y_out[:, sl], in_=y[:])

    nc.compile()
    _PROGS[S] = nc
    return nc


# ---------------- host-side input prep ----------------------------------------
def _fold_weights(core, W1, b1, W2, b2, W3, b3):
    means, std, mid, Lb, Rb = _geometry()
    base = DOM0 + core * DW
    act = _active_windows(core)
    assert len(act) <= NSLOT, f"core {core}: {len(act)} active windows"
    cspack = np.zeros((P, CSW), np.float32)
    w2pack = np.zeros((P, P * NG), np.float32)
    spack = np.zeros((NSLOT, SW), np.float32)
    sc1h = np.zeros((P, NG), np.float64)
    bi1b = np.zeros((P, NG), np.float64)
    for slot, w in enumerate(act):
        g, s = divmod(slot, 4)
        rows = slice(32 * s, 32 * s + 32)
        w1r = W1[w, 0, :].astype(np.float64)
        sc1h[rows, g] = w1r / std[w] * H
        bi1b[rows, g] = b1[w] + w1r * (base - means[w]) / std[w]
        w2pack[rows, g * P + 32 * s:g * P + 32 * s + 32] = W2[w]
        cspack[rows, O_W3 + g * NSLOT + slot] = W3[w, :, 0]
        cspack[rows, O_B2 + g] = b2[w]
        spack[slot, O_B3] = b3[w, 0]
        spack[slot, O_T1] = np.float32((base - np.float64(mid[w])) / 2.0)
        spack[slot, O_T2] = np.float32((np.float64(mid[w + 1]) - base) / 2.0)
    cspack[:, O_SC1:O_SC1 + NG] = sc1h.astype(np.float32)
    cspack[:, O_BI1:O_BI1 + NG] = bi1b.astype(np.float32)
    # cell-left x per partition (exact in fp32)
    cellx = (np.float64(base) + np.arange(P, dtype=np.float64) * H).astype(np.float32)
    cspack[:, O_XL] = cellx
    cspack[:, O_XR] = (np.float64(base)
                       + np.arange(1, P + 1, dtype=np.float64) * H).astype(np.float32)
    # window mask at knots (0.25 factor of the tanh-sigmoid identity folded in)
    kx = (np.float64(base) + np.arange(NK, dtype=np.float64) * H).astype(np.float32)
    for slot, w in enumerate(act):
        lbv = np.nextafter(Lb[w], -np.inf)
        spack[slot, O_WM:O_WM + NK] = 0.25 * ((kx > lbv) & (kx < Rb[w]))
    spack[:, O_ON] = 1.0
    # inactive slots: park the window far away so tanh args stay finite
    for slot in range(len(act), NSLOT):
        spack[slot, O_T1] = -1e4
        spack[slot, O_T2] = -1e4
    return cspack, w2pack, spack


def _prep_in_maps(inputs, S):
    x = np.asarray(inputs["x"], np.float32)
    W1 = np.asarray(inputs["W1"], np.float32)
    b1 = np.asarray(inputs["b1"], np.float32)
    W2 = np.asarray(inputs["W2"], np.float32)
    b2 = np.asarray(inputs["b2"], np.float32)
    W3 = np.asarray(inputs["W3"], np.float32)
    b3 = np.asarray(inputs["b3"], np.float32)

    ncell = NCORES * C_LOC
    cglob = np.minimum((x.astype(np.float64) * (1.0 / H)).astype(np.int64),
                       ncell - 1)
    order = np.argsort(cglob, kind="stable")
    cs = cglob[order]
    cnt = np.bincount(cglob, minlength=ncell)
    maxcnt = int(cnt.max())
    if maxcnt > S:
        raise OverflowError(maxcnt)
    starts = np.concatenate(([0], np.cumsum(cnt)))
    rank = np.arange(len(x)) - starts[cs]           # rank within own cell
    slot = cs * S + rank                            # global padded slot index

    # t = (x - cell_left)/h in [0,1), sent as fp16
    cellxg = (cglob.astype(np.float64) * H).astype(np.float32)   # exact fp32
    tval = ((x - cellxg) * np.float32(INVH)).astype(np.float16)

    in_maps = []
    for core in range(NCORES):
        tpad = np.zeros(C_LOC * S, np.float16)
        msk = (cs >= core * C_LOC) & (cs < (core + 1) * C_LOC)
        tpad[slot[msk] - core * C_LOC * S] = tval[order[msk]]
        cspack, w2pack, spack = _fold_weights(core, W1, b1, W2, b2, W3, b3)
        in_maps.append({
            "t_pts": tpad.reshape(P, S),
            "cspack": cspack,
            "w2pack": w2pack,
            "spack": spack,
        })
    return in_maps, order, slot


def _unpack(results, order, slot, n_total):
    allys = np.concatenate([r["y_out"].reshape(-1) for r in results])
    out = np.empty(n_total, np.float32)
    out[order] = allys[slot].astype(np.float32)
    return out


def kernel(**inputs) -> np.ndarray:
    from concourse.bass_utils import run_bass_kernel_spmd

    S = S_DEFAULT
    while True:
        try:
            in_maps, order, slot = _prep_in_maps(inputs, S)
            break
        except OverflowError as e:
            S = ((int(e.args[0]) + 23) // 16) * 16   # headroom, multiple of 16
    nc = _build_program(S)
    res = run_bass_kernel_spmd(nc, in_maps, list(range(NCORES)))
    return _unpack(res.results, order, slot, len(np.asarray(inputs["x"])))


# revision 12
# speedup vs baseline: 5.2244x; 1.3669x over previous
"""FBPinn forward kernel for Trainium2 (8 NeuronCores, Bass/Tile).

The module computes y(x) = tanh(x) * sum_w [win_w(x)>1e-3] * win_w(x) * MLP_w(x)
for 1M scalar points x in [0,100) -- a fixed 1D function of x. Tolerance is
rel 2e-2, so a piecewise-linear table on a coarse grid suffices (measured
~2e-3 absmax with 128 cells/core, dominated by interpolating through the
win>1e-3 mask jumps, which this kernel does NOT special-case).

Per core (12.5-wide domain slice, 128 cells, one cell per SBUF partition):
  1. phase B: evaluate the function at the 129 knots of a uniform grid using
     the <=12 active per-window MLPs (block-diagonal-packed fp16 PE matmuls,
     fp32 PSUM accumulate). All activations are Tanh -- window sigmoids use
     sigmoid(z)=(1+tanh(z/2))/2 -- so the ACT table never swaps; a dummy
     activation at t=0 prefetches it. Knot x values are generated on-chip
     (f32 iota) and folded into activation scale/bias host-side.
  2. phase C: the final sum-over-slots matmul is done twice with lhsT=term
     shifted by one knot, yielding knot values directly in partition-major
     [128,1] columns; cell records (ulo, dv) take 2 fused DVE ops.
  3. phase D: points are packed (host side) so partition p holds exactly the
     points of cell p (S slots). The host sends t=(x-cell_left)/h as fp16;
     interpolation is ONE fused tensor_scalar per chunk: y = t*dv_p + ulo_p,
     written back as fp16 (host upcasts).
DMA is 2 input issues (one f32 const pack; one f16 stream carrying w2 then
the points) + 4 output stores. Host shards points by domain across the
8 cores, packs slots, and un-permutes the outputs.
"""

import numpy as np

# ---------------- problem constants (hardcoded from the module spec) ----------
NW = 30
DOM0, DOM1 = 0.0, 100.0
OVERLAP = 0.25
NEURONS = 32
THRESH = 0.001
N = 1_000_000

NCORES = 8
P = 128                      # SBUF partitions
C_LOC = P                    # cells per core: one per partition
DW = 12.5                    # per-core domain width
H = DW / C_LOC               # cell width = 0.09765625 (exact in fp32)
INVH = C_LOC / DW
NG = 3                       # window groups of 4 per core
NSLOT = 4 * NG               # window slots per core
NK = C_LOC + 1               # knots per core (129)
NKP = 132                    # padded iota width
NCHI = 2                     # phase-D input chunks
NCHO = 4                     # phase-D output chunks
S_DEFAULT = 1104             # point slots per cell (max bin count 1084 + pad)
W2C = P * NG                 # fp16 w2 columns in the tw stream

# f32 const-pack [P, CW]: sc1h | bi1b | b2c | w3f | xl | xr | s12 | b12 | spack
# (spack region only rows 0:NSLOT are meaningful; s12/b12 rows 0:2*NSLOT)
O_SC1 = 0
O_BI1 = O_SC1 + NG
O_B2 = O_BI1 + NG
O_W3 = O_B2 + NG
O_XL = O_W3 + NSLOT * NG
O_XR = O_XL + 1
O_S12 = O_XR + 1
O_B12 = O_S12 + 1
O_WM = O_B12 + 1             # [NSLOT, NK] window mask (0.25*mask)
O_B3 = O_WM + NK
O_ON = O_B3 + 1
CW = O_ON + 1


# ---------------- geometry (host, input-independent) --------------------------
def _partition_geom():
    width = (DOM1 - DOM0) / NW
    sub = np.zeros((NW, 2), np.float32)
    for i in range(NW):
        sub[i, 0] = DOM0 if i == 0 else DOM0 + (i - OVERLAP / 2) * width
        sub[i, 1] = DOM1 if i == NW - 1 else DOM0 + (i + 1 + OVERLAP / 2) * width
    means = (sub[:, 0] + sub[:, 1]) / 2
    std = (sub[:, 1] - sub[:, 0]) / 2
    mid = np.zeros(NW + 1, np.float32)
    mid[0] = sub[0, 0]
    mid[-1] = sub[-1, 1]
    for i in range(1, NW):
        mid[i] = (sub[i - 1, 1] + sub[i, 0]) / 2
    return means.astype(np.float32), std.astype(np.float32), mid.astype(np.float32)


def _win64(l, r, x):
    return 1.0 / (1 + np.exp(-(x - l))) / (1 + np.exp(x - r))


def _bisect64(l, r, lo, hi, rising):
    for _ in range(200):
        m = 0.5 * (lo + hi)
        if (_win64(l, r, m) < THRESH) == rising:
            lo = m
        else:
            hi = m
    return 0.5 * (lo + hi)


def _refine_flip_fp32(l32, r32, b64, rising):
    """Exact fp32 x where the reference's jax-fp32 predicate win(x)>1e-3 flips.
    Returns the smallest fp32 x at which the predicate equals its right-side
    state. Falls back to the float64 bisection value if jax is unavailable."""
    try:
        import jax
        import jax.numpy as jnp

        cpu = jax.devices("cpu")[0]
        lo = np.float32(b64 - 5e-5)
        hi = np.float32(b64 + 5e-5)
        xs = np.arange(lo.view(np.int32), hi.view(np.int32) + 1,
                       dtype=np.int32).view(np.float32)
        with jax.default_device(cpu):
            win = np.asarray(
                jax.nn.sigmoid(jnp.asarray(xs) - np.float32(l32))
                * jax.nn.sigmoid(-(jnp.asarray(xs) - np.float32(r32)))
            )
        pred = win > np.float32(THRESH)
        state = pred if rising else ~pred
        if not state.any() or state.all():
            return np.float32(b64)
        k = int(np.argmax(state))
        if not state[k:].all():
            return np.float32(b64)
        return xs[k]
    except Exception:
        return np.float32(b64)


_GEOM = None


def _geometry():
    global _GEOM
    if _GEOM is not None:
        return _GEOM
    means, std, mid = _partition_geom()
    ml = mid[:-1].astype(np.float64)
    mr = mid[1:].astype(np.float64)
    Lb = np.zeros(NW, np.float32)   # window-on lower bound (exact fp32 flip)
    Rb = np.zeros(NW, np.float32)   # window-off upper bound
    for w in range(NW):
        c = 0.5 * (ml[w] + mr[w])
        l64 = _bisect64(ml[w], mr[w], ml[w] - 30, c, rising=True)
        r64 = _bisect64(ml[w], mr[w], c, mr[w] + 30, rising=False)
        Lb[w] = _refine_flip_fp32(mid[w], mid[w + 1], l64, rising=True)
        Rb[w] = _refine_flip_fp32(mid[w], mid[w + 1], r64, rising=False)
    _GEOM = (means, std, mid, Lb, Rb)
    return _GEOM


def _active_windows(core):
    means, std, mid, Lb, Rb = _geometry()
    base = DOM0 + core * DW
    return [w for w in range(NW) if (Rb[w] > base) and (Lb[w] < base + DW)]


# ---------------- bass program (built once per S, SPMD across 8 cores) --------
_PROGS = {}


def _build_program(S):
    if S in _PROGS:
        return _PROGS[S]
    from concourse import bacc, mybir, tile

    f32 = mybir.dt.float32
    f16 = mybir.dt.float16
    Act = mybir.ActivationFunctionType
    Op = mybir.AluOpType

    assert S % (NCHI * NCHO) == 0
    CHI = S // NCHI
    CHO = S // NCHO

    nc = bacc.Bacc(None, target_bir_lowering=False)

    tw_in = nc.declare_dram_parameter("tw", [P, W2C + S], f16, isOutput=False)
    cp_in = nc.declare_dram_parameter("cpack", [P, CW], f32, isOutput=False)
    y_out = nc.declare_dram_parameter("y_out", [P, S], f16, isOutput=True)

    with tile.TileContext(nc) as tc:
        with (
            tc.tile_pool(name="const", bufs=1) as cpool,
            tc.tile_pool(name="work", bufs=2) as wpool,
            tc.tile_pool(name="pts", bufs=4) as ppool,
            tc.tile_pool(name="psum", bufs=1, space="PSUM") as psum,
        ):
            # ---- prefetch the Tanh ACT table behind the const DMAs ----
            dmy = cpool.tile([1, 4], f32, tag="dmy")
            nc.vector.memset(dmy[:], 0.0)
            dmy2 = cpool.tile([1, 4], f32, tag="dmy2")
            nc.scalar.activation(out=dmy2[:], in_=dmy[:], func=Act.Tanh)

            # ---- input DMAs: 1 f32 const pack + the f16 w2/points stream ----
            cpk = cpool.tile([P, CW], f32, tag="c_cpk")
            nc.sync.dma_start(out=cpk[:], in_=cp_in[:])
            tw = cpool.tile([P, W2C + S], f16, tag="c_tw")
            nc.scalar.dma_start(out=tw[:, 0:W2C], in_=tw_in[:, 0:W2C])
            nc.gpsimd.dma_start(out=tw[:, W2C:W2C + CHI],
                                in_=tw_in[:, W2C:W2C + CHI])
            nc.scalar.dma_start(out=tw[:, W2C + CHI:W2C + S],
                                in_=tw_in[:, W2C + CHI:W2C + S])

            sc1h = cpk[:, O_SC1:O_SC1 + NG]
            bi1b = cpk[:, O_BI1:O_BI1 + NG]
            b2c = cpk[:, O_B2:O_B2 + NG]
            w3c = cpk[:, O_W3:O_W3 + NSLOT * NG]
            xl = cpk[:, O_XL:O_XL + 1]
            xr = cpk[:, O_XR:O_XR + 1]
            s12 = cpk[0:32 + NSLOT, O_S12:O_S12 + 1]
            b12 = cpk[0:32 + NSLOT, O_B12:O_B12 + 1]
            wm = cpk[0:NSLOT, O_WM:O_WM + NK]
            wm32 = cpk[32:32 + NSLOT, O_WM:O_WM + NK]
            b3c = cpk[0:NSLOT, O_B3:O_B3 + 1]
            onc = cpk[0:NSLOT, O_ON:O_ON + 1]
            w2 = tw[:, 0:W2C]
            tp = tw[:, W2C:W2C + S]

            # fp16 copies of the tiny matmul operands
            w3f = cpool.tile([P, NSLOT * NG], f16, tag="c_w3f")
            nc.vector.tensor_copy(out=w3f[:], in_=w3c)
            on16 = cpool.tile([NSLOT, 1], f16, tag="c_on16")
            nc.vector.tensor_copy(out=on16[:], in_=onc)

            # ---- knot index row (on-chip, value = column index) ----
            krow = cpool.tile([P, NKP], f32, tag="c_krf")
            nc.gpsimd.iota(krow[:], pattern=[[1, NKP]], channel_multiplier=0,
                           allow_small_or_imprecise_dtypes=True)

            # ---- phase B: knot values (single chunk, all-Tanh, fp16 mm) ----
            h2s = []
            for g in range(NG):
                h1 = wpool.tile([P, NK], f16, tag=f"h1_{g}")
                nc.scalar.activation(out=h1[:], in_=krow[:, :NK], func=Act.Tanh,
                                     bias=bi1b[:, g:g + 1], scale=sc1h[:, g:g + 1])
                h2p = psum.tile([P, NK], f32, tag=f"h2p_{g}")
                nc.tensor.matmul(out=h2p[:], lhsT=w2[:, g * P:(g + 1) * P],
                                 rhs=h1[:], start=True, stop=True)
                h2 = wpool.tile([P, NK], f16, tag=f"h2_{g}")
                nc.scalar.activation(out=h2[:], in_=h2p[:], func=Act.Tanh,
                                     bias=b2c[:, g:g + 1], scale=1.0)
                h2s.append(h2)
            pre = psum.tile([NSLOT, NK], f32, tag="pre")
            for g in range(NG):
                nc.tensor.matmul(out=pre[:], lhsT=w3f[:, g * NSLOT:(g + 1) * NSLOT],
                                 rhs=h2s[g][:], start=(g == 0), stop=(g == NG - 1))
            # window: win = 0.25*(1+tanh((x-l)/2))*(1+tanh((r-x)/2)) * mask
            # rows 0:12 of t12 are tanh(+(x-l)/2), rows 32:44 tanh(-(x-r)/2)
            # (second bank at partition 32: engine partition windows must be
            # 32-aligned)
            t12 = wpool.tile([32 + NSLOT, NK], f32, tag="t12")
            nc.scalar.activation(out=t12[:], in_=krow[0:32 + NSLOT, :NK],
                                 func=Act.Tanh, scale=s12, bias=b12)
            wbm = wpool.tile([NSLOT, NK], f32, tag="wbm")
            nc.vector.scalar_tensor_tensor(out=wbm[:], in0=t12[32:32 + NSLOT, :],
                                           scalar=1.0, in1=wm32, op0=Op.add,
                                           op1=Op.mult)
            win = wpool.tile([NSLOT, NK], f32, tag="win")
            nc.vector.scalar_tensor_tensor(out=win[:], in0=t12[0:NSLOT, :],
                                           scalar=1.0, in1=wbm[:], op0=Op.add,
                                           op1=Op.mult)
            term = wpool.tile([NSLOT, NK], f16, tag="term")
            nc.vector.scalar_tensor_tensor(out=term[:], in0=pre[:], scalar=b3c,
                                           in1=win[:], op0=Op.add, op1=Op.mult)

            # ---- phase C: partition-major knot sums + cell records ----
            vlo = psum.tile([P, 1], f32, tag="vlo")
            nc.tensor.matmul(out=vlo[:], lhsT=term[:, 0:P], rhs=on16[:],
                             start=True, stop=True)
            vhi = psum.tile([P, 1], f32, tag="vhi")
            nc.tensor.matmul(out=vhi[:], lhsT=term[:, 1:P + 1], rhs=on16[:],
                             start=True, stop=True)
            thlo = wpool.tile([P, 1], f32, tag="thlo")
            nc.scalar.activation(out=thlo[:], in_=xl, func=Act.Tanh)
            thhi = wpool.tile([P, 1], f32, tag="thhi")
            nc.scalar.activation(out=thhi[:], in_=xr, func=Act.Tanh)
            ulos = wpool.tile([P, 1], f32, tag="ulos")
            nc.vector.tensor_mul(out=ulos[:], in0=vlo[:], in1=thlo[:])
            dvs = wpool.tile([P, 1], f32, tag="dvs")
            nc.vector.scalar_tensor_tensor(out=dvs[:], in0=vhi[:], scalar=thhi[:],
                                           in1=ulos[:], op0=Op.mult,
                                           op1=Op.subtract)

            # ---- phase D: y = t*dv + ulo, fp16 in/out, one fused op/chunk ----
            oeng = [nc.sync, nc.scalar]
            for ch in range(NCHO):
                sl = slice(ch * CHO, (ch + 1) * CHO)
                y = ppool.tile([P, CHO], f16, tag="y")
                nc.vector.tensor_scalar(out=y[:], in0=tp[:, sl], scalar1=dvs[:],
                                        scalar2=ulos[:], op0=Op.mult, op1=Op.add)
                oeng[ch % 2].dma_start(out=y_out[:, sl], in_=y[:])

    nc.compile()
    _PROGS[S] = nc
    return nc


# ---------------- host-side input prep ----------------------------------------
def _fold_weights(core, W1, b1, W2, b2, W3, b3):
    means, std, mid, Lb, Rb = _geometry()
    base = DOM0 + core * DW
    act = _active_windows(core)
    assert len(act) <= NSLOT, f"core {core}: {len(act)} active windows"
    cpack = np.zeros((P, CW), np.float32)
    w2pack = np.zeros((P, W2C), np.float16)
    sc1h = np.zeros((P, NG), np.float64)
    bi1b = np.zeros((P, NG), np.float64)
    for slot, w in enumerate(act):
        g, s = divmod(slot, 4)
        rows = slice(32 * s, 32 * s + 32)
        w1r = W1[w, 0, :].astype(np.float64)
        sc1h[rows, g] = w1r / std[w] * H
        bi1b[rows, g] = b1[w] + w1r * (base - means[w]) / std[w]
        w2pack[rows, g * P + 32 * s:g * P + 32 * s + 32] = W2[w].astype(np.float16)
        cpack[rows, O_W3 + g * NSLOT + slot] = W3[w, :, 0]
        cpack[rows, O_B2 + g] = b2[w]
        cpack[slot, O_B3] = b3[w, 0]
        cpack[slot, O_B12] = np.float32((base - np.float64(mid[w])) / 2.0)
        cpack[32 + slot, O_B12] = np.float32((np.float64(mid[w + 1]) - base) / 2.0)
    cpack[:, O_SC1:O_SC1 + NG] = sc1h.astype(np.float32)
    cpack[:, O_BI1:O_BI1 + NG] = bi1b.astype(np.float32)
    cpack[0:NSLOT, O_S12] = H / 2
    cpack[32:32 + NSLOT, O_S12] = -H / 2
    # cell-left/right x per partition (exact in fp32)
    cpack[:, O_XL] = (np.float64(base)
                      + np.arange(P, dtype=np.float64) * H).astype(np.float32)
    cpack[:, O_XR] = (np.float64(base)
                      + np.arange(1, P + 1, dtype=np.float64) * H).astype(np.float32)
    # window mask at knots (0.25 factor of the tanh-sigmoid identity folded in)
    kx = (np.float64(base) + np.arange(NK, dtype=np.float64) * H).astype(np.float32)
    for slot, w in enumerate(act):
        lbv = np.nextafter(Lb[w], -np.inf)
        mrow = 0.25 * ((kx > lbv) & (kx < Rb[w]))
        cpack[slot, O_WM:O_WM + NK] = mrow
        cpack[32 + slot, O_WM:O_WM + NK] = mrow   # copy at partition base 32
    cpack[0:NSLOT, O_ON] = 1.0
    # inactive slots (and the 12:32 alignment gap): park the window far away
    for slot in range(len(act), NSLOT):
        cpack[slot, O_B12] = -1e4
        cpack[32 + slot, O_B12] = -1e4
    cpack[NSLOT:32, O_B12] = -1e4
    return cpack, w2pack


def _prep_in_maps(inputs, S):
    x = np.asarray(inputs["x"], np.float32)
    W1 = np.asarray(inputs["W1"], np.float32)
    b1 = np.asarray(inputs["b1"], np.float32)
    W2 = np.asarray(inputs["W2"], np.float32)
    b2 = np.asarray(inputs["b2"], np.float32)
    W3 = np.asarray(inputs["W3"], np.float32)
    b3 = np.asarray(inputs["b3"], np.float32)

    ncell = NCORES * C_LOC
    cglob = np.minimum((x.astype(np.float64) * (1.0 / H)).astype(np.int64),
                       ncell - 1)
    order = np.argsort(cglob, kind="stable")
    cs = cglob[order]
    cnt = np.bincount(cglob, minlength=ncell)
    maxcnt = int(cnt.max())
    if maxcnt > S:
        raise OverflowError(maxcnt)
    starts = np.concatenate(([0], np.cumsum(cnt)))
    rank = np.arange(len(x)) - starts[cs]           # rank within own cell
    slot = cs * S + rank                            # global padded slot index

    # t = (x - cell_left)/h in [0,1), sent as fp16
    cellxg = (cglob.astype(np.float64) * H).astype(np.float32)   # exact fp32
    tval = ((x - cellxg) * np.float32(INVH)).astype(np.float16)

    in_maps = []
    for core in range(NCORES):
        tw = np.zeros((P, W2C + S), np.float16)
        msk = (cs >= core * C_LOC) & (cs < (core + 1) * C_LOC)
        loc = slot[msk] - core * C_LOC * S
        tw[loc // S, W2C + loc % S] = tval[order[msk]]
        cpack, w2pack = _fold_weights(core, W1, b1, W2, b2, W3, b3)
        tw[:, 0:W2C] = w2pack
        in_maps.append({
            "tw": tw,
            "cpack": cpack,
        })
    return in_maps, order, slot


def _unpack(results, order, slot, n_total):
    allys = np.concatenate([r["y_out"].reshape(-1) for r in results])
    out = np.empty(n_total, np.float32)
    out[order] = allys[slot].astype(np.float32)
    return out


def kernel(**inputs) -> np.ndarray:
    from concourse.bass_utils import run_bass_kernel_spmd

    S = S_DEFAULT
    while True:
        try:
            in_maps, order, slot = _prep_in_maps(inputs, S)
            break
        except OverflowError as e:
            S = ((int(e.args[0]) + 23) // 16) * 16   # headroom, multiple of 16
    nc = _build_program(S)
    res = run_bass_kernel_spmd(nc, in_maps, list(range(NCORES)))
    return _unpack(res.results, order, slot, len(np.asarray(inputs["x"])))
